# revision 75
# baseline (speedup 1.0000x reference)
"""Multi-head self-attention (B=8, T=2048, C=192, H=6, HS=32) on 8 TRN2 cores.

Sharding: data-parallel over batch — core i computes batch element i fully
on-chip (no collectives). Host pre-transposes x and packs weights so the
device does zero transposes.

Per-core pipeline (v1.5 — three-way exp split):
  qT/kT [d, t] = W^T @ xT                   (PE; PSUM->SBUF copies spread)
  v_aug [s, (h|1)]                          (PE via the y bank in tc0)
  S^T   [s, t] = kT_h^T @ qT_h              (PE, K=32, tile_position per head)
  P^T = exp(S/sqrt(HS)) split across THREE engines per s-tile:
        ACT head-pair [128,1024] exact exp; DVE head-pair Schraudolph
        (bits = trunc(S1*x+S2) as int16 -> bf16); Pool two singles same trick
  O[t, d|rowsum] += P^T_tile^T @ v_aug      (PE, free dim 33 per head)
  O' = O * recip(rowsum)                    (DVE recip + Pool broadcast mul)
  O'^T via DMA-engine transpose (XBAR), no PE/copy cost
  y[t, c] = O'^T.T @ Wp + bias              (PE, K=192 in 2 chunks + bias row)
"""

import numpy as np
import ml_dtypes
from contextlib import ExitStack

import concourse.bass as bass
import concourse.tile as tile
from concourse import bacc, mybir
from concourse.bass_utils import run_bass_kernel_spmd

B, T, C = 8, 2048, 192
H, HS = 6, 32
P = 128
TCH = 512            # t-chunk width per head
NT = T // TCH        # 4
NS = T // P          # 16 s-tiles
E1 = HS + 1          # 33: per-head AV free dim (32 d + rowsum)
SCALE = 1.0 / float(np.sqrt(HS))
BF16 = mybir.dt.bfloat16
F32 = mybir.dt.float32
I16 = mybir.dt.int16
FP8 = mybir.dt.float8e4
Exp = mybir.ActivationFunctionType.Exp
Alu = mybir.AluOpType
DR = mybir.MatmulPerfMode.DoubleRow

# Schraudolph bf16-bits exp: bf16(trunc(S1*x + S2)) ~ G*exp(SCALE*x).
# S2 is shifted down so the common factor G ~ exp(-3.80): that keeps the
# fp8e4m3 P tiles produced by ACT (true exp, same factor via BIAS_ACT)
# strictly below the e4m3 max of 240 for this data's z range (max ~9.1).
# G cancels exactly in the softmax normalization.
S1 = float((128.0 / np.log(2.0)) * SCALE)
S2 = 15554.0
BIAS_ACT = float(np.log(2.0) * (S2 / 128.0 - 127.0) + 0.04)

_CACHE = {}


def build_nc():
    nc = bacc.Bacc()
    xT = nc.declare_dram_parameter("xT", [C, T], BF16, isOutput=False)
    wq = nc.declare_dram_parameter("wq", [C, H * HS], BF16, isOutput=False)
    wk = nc.declare_dram_parameter("wk", [C, H * HS], BF16, isOutput=False)
    wv = nc.declare_dram_parameter("wv", [C, H * HS], BF16, isOutput=False)
    wp = nc.declare_dram_parameter("wp", [H * HS, C], BF16, isOutput=False)
    bp = nc.declare_dram_parameter("bp", [1, C], BF16, isOutput=False)
    out = nc.declare_dram_parameter("out", [T, C], F32, isOutput=True)

    with tile.TileContext(nc) as tc, ExitStack() as ctx:
        singles = ctx.enter_context(tc.tile_pool(name="singles", bufs=1))
        qk_pool = ctx.enter_context(tc.tile_pool(name="qk", bufs=1))
        ptA_pool = ctx.enter_context(tc.tile_pool(name="ptA", bufs=5))
        ptD_pool = ctx.enter_context(tc.tile_pool(name="ptD", bufs=8))
        ptP_pool = ctx.enter_context(tc.tile_pool(name="ptP", bufs=10))
        post_pool = ctx.enter_context(tc.tile_pool(name="post", bufs=6))
        oT_pool = ctx.enter_context(tc.tile_pool(name="oTp", bufs=4))
        ysb_pool = ctx.enter_context(tc.tile_pool(name="ysb", bufs=4))

        # ---------------- load inputs ----------------
        # DMA triggers cost ~500ns on the issuing engine's queue, so the
        # startup-critical loads are spread across the SP and DVE queues;
        # ACT carries only the exp table load, Pool the off-path loads.
        w_sb = {}
        for name, dram, eng in (("q", wq, nc.sync), ("k", wk, nc.gpsimd)):
            a = singles.tile([P, H * HS], BF16, name=f"w{name}a")
            eng.dma_start(a, dram[0:P, :])
            b = singles.tile([C - P, H * HS], BF16, name=f"w{name}b")
            eng.dma_start(b, dram[P:C, :])
            w_sb[name] = (a, b)
        xT_a = singles.tile([P, T], BF16)
        nc.sync.dma_start(xT_a[:, 0:TCH], xT[0:P, 0:TCH])
        xT_b = singles.tile([C - P, T], BF16)
        nc.gpsimd.dma_start(xT_b[:, 0:TCH], xT[P:C, 0:TCH])
        ones1 = singles.tile([1, P], BF16)
        nc.gpsimd.memset(ones1, 1.0)
        nc.gpsimd.dma_start(xT_a[:, TCH:T], xT[0:P, TCH:T])
        nc.sync.dma_start(xT_b[:, TCH:T], xT[P:C, TCH:T])
        wva = singles.tile([P, H * HS], BF16, name="wva")
        nc.gpsimd.dma_start(wva, wv[0:P, :])
        wvb = singles.tile([C - P, H * HS], BF16, name="wvb")
        nc.gpsimd.dma_start(wvb, wv[P:C, :])
        w_sb["v"] = (wva, wvb)

        wp_a = singles.tile([P, C], BF16, name="wpa")
        nc.gpsimd.dma_start(wp_a, wp[0:P, :])
        # wp_b data parked at partitions 64..128 so the K=64 chunk of the
        # output projection can use lhsT/rhs at matching base partition 64
        wpb_sb = singles.tile([P, C], BF16, name="wpb")
        nc.gpsimd.dma_start(wpb_sb[C - P:P, :], wp[P:H * HS, :])
        bp_sb = singles.tile([1, C], BF16)
        nc.gpsimd.dma_start(bp_sb, bp[:, :])
        # preload exp activation table (ACT queue is otherwise idle early)
        warm = singles.tile([1, P], BF16, name="warm")
        nc.scalar.activation(warm, ones1, Exp)
        # per-partition bias operand for the fp8 exp tiles
        bias_act = singles.tile([P, 1], F32, name="biasact")
        nc.gpsimd.memset(bias_act, BIAS_ACT)

        # v_aug: [s, si*(h|1)]; only the rowsum ones-columns need memset,
        # the value columns are fully written by the v copies
        v_aug = singles.tile([P, NS * H * E1], BF16, name="vaug")
        va_r = v_aug.rearrange("p (s h e) -> p s h e", s=NS, h=H)
        nc.gpsimd.memset(va_r[:, :, :, HS], 1.0)

        # ---------------- qT/kT destinations ----------------
        qT_a = qk_pool.tile([P, T], BF16)       # heads 0..3, d-major
        qT_b = qk_pool.tile([C - P, T], BF16)   # heads 4,5
        kT_a = qk_pool.tile([P, T], BF16)
        kT_b = qk_pool.tile([C - P, T], BF16)

        def hsrc(h):
            if h < 4:
                return kT_a, qT_a, HS * h
            return kT_b, qT_b, HS * (h - 4)

        copy_eng = {
            "a": lambda d, s: nc.scalar.copy(d, s),
            "d": lambda d, s: nc.vector.tensor_copy(d, s),
            "p": lambda d, s: nc.vector.tensor_copy(d, s),
        }

        with (
            tc.tile_pool(name="pstA", bufs=2, space="PSUM") as pstA_pool,
            tc.tile_pool(name="pstD", bufs=2, space="PSUM") as pstD_pool,
            tc.tile_pool(name="pav", bufs=1, space="PSUM") as pav_pool,
        ):
            def proj_pair(proj, c0, eng1, eng2):
                """both dlo chunks of one [*, TCH] column range of qT/kT."""
                ps = pstA_pool.tile([P, 2 * TCH], F32, name="prj", tag="stA")
                dst_a, dst_b = ((qT_a, qT_b) if proj == "q" else (kT_a, kT_b))
                wa, wb = w_sb[proj]
                nc.tensor.matmul(ps[:, 0:TCH], wa[:, 0:P],
                                 xT_a[:, c0:c0 + TCH], start=True, stop=False)
                nc.tensor.matmul(ps[:, 0:TCH], wb[:, 0:P],
                                 xT_b[:, c0:c0 + TCH], start=False, stop=True)
                nc.tensor.matmul(ps[0:C - P, TCH:2 * TCH], wa[:, P:H * HS],
                                 xT_a[:, c0:c0 + TCH], start=True, stop=False)
                nc.tensor.matmul(ps[0:C - P, TCH:2 * TCH], wb[:, P:H * HS],
                                 xT_b[:, c0:c0 + TCH], start=False, stop=True)
                copy_eng[eng1](dst_a[:, c0:c0 + TCH], ps[:, 0:TCH])
                copy_eng[eng2](dst_b[:, c0:c0 + TCH],
                               ps[0:C - P, TCH:2 * TCH])

            def v_chunk(si, eng):
                """v_aug values for one s-tile, via the (idle in tc0) y bank."""
                s0 = si * P
                wva, wvb = w_sb["v"]
                ps = pstD_pool.tile([P, TCH], F32, name="pyv", tag="stD")
                nc.tensor.matmul(ps[:, 0:H * HS], xT_a[:, s0:s0 + P], wva,
                                 start=True, stop=False)
                nc.tensor.matmul(ps[:, 0:H * HS], xT_b[:, s0:s0 + P], wvb,
                                 start=False, stop=True)
                ps_r = ps[:, 0:H * HS].rearrange("p (h d) -> p h d", h=H)
                copy_eng[eng](va_r[:, si, :, 0:HS], ps_r)

            recs = {}

            def post_norm(av, tt):
                """normalize one t-tile: O' = O * recip(rowsum)."""
                av_t = av[tt // 2].rearrange("p (u h e) -> p u h e", u=2, h=H)
                u = tt % 2
                if u == 0:  # one reciprocal covers both u-slots of the bank
                    rec = post_pool.tile([P, 2 * H], F32, name="rec", tag="rec")
                    nc.vector.reciprocal(
                        rec.rearrange("p (u h) -> p u h", u=2),
                        av_t[:, :, :, HS])
                    recs[tt // 2] = rec
                rec = recs[tt // 2].rearrange("p (u h) -> p u h", u=2)[:, u, :]
                onorm = post_pool.tile([P, H * HS], BF16,
                                       name="onorm", tag="onorm")
                on_r = onorm.rearrange("p (h e) -> p h e", h=H)
                nc.vector.tensor_tensor(
                    on_r, av_t[:, u, :, 0:HS],
                    rec.unsqueeze(2).to_broadcast([P, H, HS]),
                    Alu.mult)
                return onorm

            def post_issue(onorm):
                """phase 1: start the two xbar transposes of one t-tile."""
                oT1 = oT_pool.tile([P, P], BF16, name="oT1", tag="oT1")
                nc.sync.dma_start_transpose(oT1, onorm[:, 0:P])
                oT2 = oT_pool.tile([P, P], BF16, name="oT2", tag="oT2")
                nc.sync.dma_start_transpose(oT2, onorm[:, C - P:C])
                return oT1, oT2

            def post_proj(tc0, oTs, tt, pool=None):
                """phase 2: project and store one t-tile (oTs ready-ish, so
                the borrowed psum slot is held briefly)."""
                oT1, oT2 = oTs
                if pool is None or pool is pstD_pool:
                    ps = pstD_pool.tile([P, TCH], F32, name="pyy", tag="stD")
                else:
                    ps = pstA_pool.tile([P, 2 * TCH], F32,
                                        name="pyy2", tag="stA")
                nc.tensor.matmul(ps[:, 0:C], ones1, bp_sb,
                                 start=True, stop=False)
                nc.tensor.matmul(ps[:, 0:C], oT1, wp_a,
                                 start=False, stop=False)
                nc.tensor.matmul(ps[:, 0:C], oT2[C - P:P, :],
                                 wpb_sb[C - P:P, :],
                                 start=False, stop=True)
                ysb = ysb_pool.tile([P, C], F32, name="ysbt", tag="ysbt")
                nc.vector.tensor_copy(ysb, ps[:, 0:C])
                oq = nc.sync if tt % 2 == 0 else nc.gpsimd
                oq.dma_start(out[tc0 + tt * P:tc0 + (tt + 1) * P, :], ysb)

            def emit_av(av, si, ptiles, heads, tts=tuple(range(NT))):
                """bf16 AV for the non-ACT heads of one s-tile."""
                for h in heads:
                    for tt in tts:
                        av_t = av[tt // 2].rearrange(
                            "p (u h e) -> p u h e", u=2, h=H)
                        nc.tensor.matmul(
                            av_t[:, tt % 2, h, :],
                            ptiles[h][:, tt * P:(tt + 1) * P],
                            va_r[:, si, h, :],
                            start=(si == 0 and h == heads[0]
                                   and tt % 2 == 0),
                            stop=(si == NS - 1),
                            skip_group_check=True)


            def qkt(st_dst, h, si, tc0):
                kT_t, qT_t, pb = hsrc(h)
                nc.tensor.matmul(
                    st_dst, kT_t[pb:pb + HS, si * P:si * P + P],
                    qT_t[pb:pb + HS, tc0:tc0 + TCH],
                    start=True, stop=True, tile_position=(pb, 0))

            # head-group -> engine rotation per si: each group of 2 heads goes
            # to one of ACT / DVE / Pool(2 singles)
            GROUPS = ((0, 1), (2, 3), (4, 5))

            # prologue. k-mini first: si-0 QKTs only need kT cols 0:128, so a
            # tiny k chunk unblocks them well before the full k pair lands
            kmini = pstD_pool.tile([P, TCH], F32, name="kmini", tag="stD")
            wka, wkb = w_sb["k"]
            nc.tensor.matmul(kmini[:, 0:P], wka[:, 0:P], xT_a[:, 0:P],
                             start=True, stop=False)
            nc.tensor.matmul(kmini[:, 0:P], wkb[:, 0:P], xT_b[:, 0:P],
                             start=False, stop=True)
            nc.tensor.matmul(kmini[0:C - P, P:2 * P], wka[:, P:H * HS],
                             xT_a[:, 0:P], start=True, stop=False)
            nc.tensor.matmul(kmini[0:C - P, P:2 * P], wkb[:, P:H * HS],
                             xT_b[:, 0:P], start=False, stop=True)
            nc.vector.tensor_copy(kT_a[:, 0:P], kmini[:, 0:P])
            nc.vector.tensor_copy(kT_b[:, 0:P], kmini[0:C - P, P:2 * P])
            # q pair with copies split across engines for latency
            q_ps = pstA_pool.tile([P, 2 * TCH], F32, name="qprl", tag="stA")
            wqa, wqb = w_sb["q"]
            nc.tensor.matmul(q_ps[:, 0:TCH], wqa[:, 0:P],
                             xT_a[:, 0:TCH], start=True, stop=False)
            nc.tensor.matmul(q_ps[:, 0:TCH], wqb[:, 0:P],
                             xT_b[:, 0:TCH], start=False, stop=True)
            nc.tensor.matmul(q_ps[0:C - P, TCH:2 * TCH], wqa[:, P:H * HS],
                             xT_a[:, 0:TCH], start=True, stop=False)
            nc.tensor.matmul(q_ps[0:C - P, TCH:2 * TCH], wqb[:, P:H * HS],
                             xT_b[:, 0:TCH], start=False, stop=True)
            nc.scalar.copy(qT_a[:, 0:2 * P], q_ps[:, 0:2 * P])
            nc.vector.tensor_copy(qT_a[:, 2 * P:TCH], q_ps[:, 2 * P:TCH])
            nc.vector.tensor_copy(qT_b[:, 0:TCH],
                                  q_ps[0:C - P, TCH:2 * TCH])
            # k for the rest of the first s-range (cols 128:512 — the kmini
            # already covered 0:128)
            k_ps = pstA_pool.tile([P, 2 * TCH], F32, name="kprl", tag="stA")
            nc.tensor.matmul(k_ps[:, P:TCH], wka[:, 0:P],
                             xT_a[:, P:TCH], start=True, stop=False)
            nc.tensor.matmul(k_ps[:, P:TCH], wkb[:, 0:P],
                             xT_b[:, P:TCH], start=False, stop=True)
            nc.tensor.matmul(k_ps[0:C - P, TCH + P:2 * TCH], wka[:, P:H * HS],
                             xT_a[:, P:TCH], start=True, stop=False)
            nc.tensor.matmul(k_ps[0:C - P, TCH + P:2 * TCH], wkb[:, P:H * HS],
                             xT_b[:, P:TCH], start=False, stop=True)
            nc.scalar.copy(kT_a[:, P:TCH], k_ps[:, P:TCH])
            nc.vector.tensor_copy(kT_b[:, P:TCH],
                                  k_ps[0:C - P, TCH + P:2 * TCH])

            # deferred projection chunks, keyed by (tc index, si)
            deferred = {}
            for i, c0 in enumerate((TCH, 2 * TCH, 3 * TCH)):
                deferred.setdefault((0, 3 * i + 1), []).append(
                    ("k", c0, "a", "d"))
            for tci in range(NT - 1):
                deferred.setdefault((tci, 9), []).append(
                    ("q", (tci + 1) * TCH, "a", "d"))

            prev1 = prev2 = None  # (av, si, ptiles, heads): AV lags 2 si
            dr_q = []             # [(j, ptJ, ga)] DoubleRow AV queue
            cur_ptJ = None
            pending = None  # (tc0, onorms) awaiting post_proj
            av = None
            for tci, tc0 in enumerate(range(0, T, TCH)):
                old_av = av
                av = [pav_pool.tile([P, 2 * H * E1], F32,
                                    name=f"av{i}", tag=f"av{i}")
                      for i in range(2)]
                for si in range(NS):
                    rot = (si // 2) % 3  # per s-tile PAIR so the ACT group
                    # (fp8 DoubleRow) is uniform across the pair
                    eng_of = {g: ("a", "d", "p")[(g + rot) % 3]
                              for g in range(3)}
                    ptiles = [None] * H
                    ga = next(g for g in range(3) if eng_of[g] == "a")
                    gd = next(g for g in range(3) if eng_of[g] == "d")
                    gp = next(g for g in range(3) if eng_of[g] == "p")
                    heads_bf = list(range(H))

                    # ACT group -> one [128,1024] exp into the fp8 pair
                    # container (DoubleRow AV); stA pair tiles bufs=2
                    h_p0, h_p1 = GROUPS[gp]
                    stA = pstA_pool.tile([P, 2 * TCH], F32, name="stA",
                                         tag="stA")
                    for half, h in enumerate(GROUPS[ga]):
                        qkt(stA[:, half * TCH:(half + 1) * TCH], h, si, tc0)
                    ptpA = ptA_pool.tile([P, 2 * TCH], BF16, name="ptpA",
                                         tag="ptj")
                    nc.scalar.activation(ptpA, stA, Exp, scale=SCALE,
                                         bias=bias_act)
                    ptiles[GROUPS[ga][0]] = ptpA[:, 0:TCH]
                    ptiles[GROUPS[ga][1]] = ptpA[:, TCH:2 * TCH]

                    # gp group: ACT bf16 pair on 7 of 8 si, DVE singles else
                    if si % 8 != 7 and si % 32 != 3:
                        stG = pstA_pool.tile([P, 2 * TCH], F32, name="stG",
                                             tag="stA")
                        for half, h in enumerate(GROUPS[gp]):
                            qkt(stG[:, half * TCH:(half + 1) * TCH],
                                h, si, tc0)
                        ptpG = ptP_pool.tile([P, 2 * TCH], BF16, name="ptpG",
                                             tag="ptp")
                        nc.scalar.activation(ptpG, stG, Exp, scale=SCALE,
                                             bias=bias_act)
                        ptiles[h_p0] = ptpG[:, 0:TCH]
                        ptiles[h_p1] = ptpG[:, TCH:2 * TCH]
                    else:
                        for h in GROUPS[gp]:
                            stG = pstD_pool.tile([P, TCH], F32, name="stG1",
                                                 tag="stD")
                            qkt(stG, h, si, tc0)
                            ptpG = ptP_pool.tile([P, 2 * TCH], BF16,
                                                 name="ptpG", tag="ptp")
                            nc.vector.tensor_scalar(
                                ptpG.bitcast(I16)[:, 0:TCH], stG,
                                S1, S2, Alu.mult, Alu.add)
                            ptiles[h] = ptpG[:, 0:TCH]

                    # DVE singles, double-buffered via pstD bufs=2
                    for h in GROUPS[gd]:
                        stD = pstD_pool.tile([P, TCH], F32, name="stD",
                                             tag="stD")
                        qkt(stD, h, si, tc0)
                        ptpD = ptD_pool.tile([P, TCH], BF16, name="ptpD",
                                             tag="ptpd")
                        nc.vector.tensor_scalar(ptpD.bitcast(I16), stD,
                                                S1, S2, Alu.mult, Alu.add)
                        ptiles[h] = ptpD

                    for args in deferred.get((tci, si), ()):
                        proj_pair(*args)
                    if tci == 0:
                        v_chunk(si, "d")
                    if si == 0 and old_av is not None:
                        # flush prev tc: last two bf16 s-tiles + last DR pair
                        emit_av(old_av, NS - 2, prev2[2], prev2[3])
                        emit_av(old_av, NS - 1, prev1[2], prev1[3])
                        prev2 = prev1 = None
                        pending = (tc0 - TCH,
                                   [post_norm(old_av, t) for t in (0, 1)])
                    if si == 1 and pending is not None and len(pending[1]) == 2:
                        pending[1].extend(post_norm(old_av, t) for t in (2, 3))
                    if si % 4 == 2 and pending is not None:
                        post_proj(pending[0],
                                  post_issue(pending[1][si // 4]), si // 4)
                        if si // 4 == NT - 1:
                            pending = None
                    if prev2 is not None:
                        emit_av(av, prev2[1], prev2[2], prev2[3])
                    prev2 = prev1
                    prev1 = (av, si, ptiles, heads_bf)

            # tail: drain av bank 0 fully first so its norms/projections
            # overlap the remaining AV matmuls of bank 1
            tail_pool = (None, pstA_pool, pstD_pool, None)
            emit_av(av, NS - 2, prev2[2], prev2[3], tts=(0, 1))
            emit_av(av, NS - 1, prev1[2], prev1[3], tts=(0, 1))
            oT0 = post_issue(post_norm(av, 0))
            oT1t = post_issue(post_norm(av, 1))
            post_proj(tc0, oT0, 0, pool=tail_pool[0])
            emit_av(av, NS - 2, prev2[2], prev2[3], tts=(2, 3))
            emit_av(av, NS - 1, prev1[2], prev1[3], tts=(2, 3))
            post_proj(tc0, oT1t, 1, pool=tail_pool[1])
            oT2t = post_issue(post_norm(av, 2))
            oT3t = post_issue(post_norm(av, 3))
            post_proj(tc0, oT2t, 2, pool=tail_pool[2])
            post_proj(tc0, oT3t, 3, pool=tail_pool[3])

    nc.compile()
    return nc


def _get_nc():
    if "nc" not in _CACHE:
        _CACHE["nc"] = build_nc()
    return _CACHE["nc"]


def make_in_maps(x, Wq, Wk, Wv, Wproj, bproj):
    bf = ml_dtypes.bfloat16
    f8 = ml_dtypes.float8_e4m3
    x = np.asarray(x, np.float32)
    pack = lambda w: np.ascontiguousarray(
        np.transpose(np.asarray(w, np.float32), (1, 0, 2)).reshape(C, H * HS)
    ).astype(bf)
    wq, wk, wv = pack(Wq), pack(Wk), pack(Wv)
    wp = np.ascontiguousarray(
        np.asarray(Wproj, np.float32).reshape(H * HS, C)).astype(bf)
    bp = np.asarray(bproj, np.float32).reshape(1, C).astype(bf)
    maps = []
    for i in range(B):
        xti = np.ascontiguousarray(x[i].T).astype(bf)
        maps.append({"xT": xti, "wq": wq, "wk": wk, "wv": wv,
                     "wp": wp, "bp": bp})
    return maps


def run(inputs, trace=False, **kw):
    nc = _get_nc()
    in_maps = make_in_maps(**inputs)
    res = run_bass_kernel_spmd(nc, in_maps, core_ids=list(range(B)),
                               trace=trace, **kw)
    y = np.stack([np.asarray(res.results[i]["out"], np.float32)
                  for i in range(B)], axis=0)
    return y, res


def kernel(**inputs):
    y, _ = run(inputs, trace=False)
    return y


# revision 77
# speedup vs baseline: 1.0022x; 1.0022x over previous
"""Multi-head self-attention (B=8, T=2048, C=192, H=6, HS=32) on 8 TRN2 cores.

Sharding: data-parallel over batch — core i computes batch element i fully
on-chip (no collectives). Host pre-transposes x and packs weights so the
device does zero transposes.

Per-core pipeline (v1.5 — three-way exp split):
  qT/kT [d, t] = W^T @ xT                   (PE; PSUM->SBUF copies spread)
  v_aug [s, (h|1)]                          (PE via the y bank in tc0)
  S^T   [s, t] = kT_h^T @ qT_h              (PE, K=32, tile_position per head)
  P^T = exp(S/sqrt(HS)) split across THREE engines per s-tile:
        ACT head-pair [128,1024] exact exp; DVE head-pair Schraudolph
        (bits = trunc(S1*x+S2) as int16 -> bf16); Pool two singles same trick
  O[t, d|rowsum] += P^T_tile^T @ v_aug      (PE, free dim 33 per head)
  O' = O * recip(rowsum)                    (DVE recip + Pool broadcast mul)
  O'^T via DMA-engine transpose (XBAR), no PE/copy cost
  y[t, c] = O'^T.T @ Wp + bias              (PE, K=192 in 2 chunks + bias row)
"""

import numpy as np
import ml_dtypes
from contextlib import ExitStack

import concourse.bass as bass
import concourse.tile as tile
from concourse import bacc, mybir
from concourse.bass_utils import run_bass_kernel_spmd

B, T, C = 8, 2048, 192
H, HS = 6, 32
P = 128
TCH = 512            # t-chunk width per head
NT = T // TCH        # 4
NS = T // P          # 16 s-tiles
E1 = HS + 1          # 33: per-head AV free dim (32 d + rowsum)
SCALE = 1.0 / float(np.sqrt(HS))
BF16 = mybir.dt.bfloat16
F32 = mybir.dt.float32
I16 = mybir.dt.int16
FP8 = mybir.dt.float8e4
Exp = mybir.ActivationFunctionType.Exp
Alu = mybir.AluOpType
DR = mybir.MatmulPerfMode.DoubleRow

# Schraudolph bf16-bits exp: bf16(trunc(S1*x + S2)) ~ G*exp(SCALE*x).
# S2 is shifted down so the common factor G ~ exp(-3.80): that keeps the
# fp8e4m3 P tiles produced by ACT (true exp, same factor via BIAS_ACT)
# strictly below the e4m3 max of 240 for this data's z range (max ~9.1).
# G cancels exactly in the softmax normalization.
S1 = float((128.0 / np.log(2.0)) * SCALE)
S2 = 15554.0
BIAS_ACT = float(np.log(2.0) * (S2 / 128.0 - 127.0) + 0.04)

_CACHE = {}


def build_nc():
    nc = bacc.Bacc()
    xT = nc.declare_dram_parameter("xT", [C, T], BF16, isOutput=False)
    wq = nc.declare_dram_parameter("wq", [C, H * HS], BF16, isOutput=False)
    wk = nc.declare_dram_parameter("wk", [C, H * HS], BF16, isOutput=False)
    wv = nc.declare_dram_parameter("wv", [C, H * HS], BF16, isOutput=False)
    wp = nc.declare_dram_parameter("wp", [H * HS, C], BF16, isOutput=False)
    bp = nc.declare_dram_parameter("bp", [1, C], BF16, isOutput=False)
    out = nc.declare_dram_parameter("out", [T, C], F32, isOutput=True)

    with tile.TileContext(nc) as tc, ExitStack() as ctx:
        singles = ctx.enter_context(tc.tile_pool(name="singles", bufs=1))
        qk_pool = ctx.enter_context(tc.tile_pool(name="qk", bufs=1))
        ptA_pool = ctx.enter_context(tc.tile_pool(name="ptA", bufs=7))
        ptD_pool = ctx.enter_context(tc.tile_pool(name="ptD", bufs=14))
        ptP_pool = ctx.enter_context(tc.tile_pool(name="ptP", bufs=14))
        post_pool = ctx.enter_context(tc.tile_pool(name="post", bufs=6))
        oT_pool = ctx.enter_context(tc.tile_pool(name="oTp", bufs=4))
        ysb_pool = ctx.enter_context(tc.tile_pool(name="ysb", bufs=4))

        # ---------------- load inputs ----------------
        # DMA triggers cost ~500ns on the issuing engine's queue, so the
        # startup-critical loads are spread across the SP and DVE queues;
        # ACT carries only the exp table load, Pool the off-path loads.
        w_sb = {}
        for name, dram, eng in (("q", wq, nc.sync), ("k", wk, nc.gpsimd)):
            a = singles.tile([P, H * HS], BF16, name=f"w{name}a")
            eng.dma_start(a, dram[0:P, :])
            b = singles.tile([C - P, H * HS], BF16, name=f"w{name}b")
            eng.dma_start(b, dram[P:C, :])
            w_sb[name] = (a, b)
        xT_a = singles.tile([P, T], BF16)
        nc.sync.dma_start(xT_a[:, 0:TCH], xT[0:P, 0:TCH])
        xT_b = singles.tile([C - P, T], BF16)
        nc.gpsimd.dma_start(xT_b[:, 0:TCH], xT[P:C, 0:TCH])
        ones1 = singles.tile([1, P], BF16)
        nc.gpsimd.memset(ones1, 1.0)
        nc.gpsimd.dma_start(xT_a[:, TCH:T], xT[0:P, TCH:T])
        nc.sync.dma_start(xT_b[:, TCH:T], xT[P:C, TCH:T])
        wva = singles.tile([P, H * HS], BF16, name="wva")
        nc.gpsimd.dma_start(wva, wv[0:P, :])
        wvb = singles.tile([C - P, H * HS], BF16, name="wvb")
        nc.gpsimd.dma_start(wvb, wv[P:C, :])
        w_sb["v"] = (wva, wvb)

        wp_a = singles.tile([P, C], BF16, name="wpa")
        nc.gpsimd.dma_start(wp_a, wp[0:P, :])
        # wp_b data parked at partitions 64..128 so the K=64 chunk of the
        # output projection can use lhsT/rhs at matching base partition 64
        wpb_sb = singles.tile([P, C], BF16, name="wpb")
        nc.gpsimd.dma_start(wpb_sb[C - P:P, :], wp[P:H * HS, :])
        bp_sb = singles.tile([1, C], BF16)
        nc.gpsimd.dma_start(bp_sb, bp[:, :])
        # preload exp activation table (ACT queue is otherwise idle early)
        warm = singles.tile([1, P], BF16, name="warm")
        nc.scalar.activation(warm, ones1, Exp)
        # per-partition bias operand for the fp8 exp tiles
        bias_act = singles.tile([P, 1], F32, name="biasact")
        nc.gpsimd.memset(bias_act, BIAS_ACT)

        # v_aug: [s, si*(h|1)]; only the rowsum ones-columns need memset,
        # the value columns are fully written by the v copies
        v_aug = singles.tile([P, NS * H * E1], BF16, name="vaug")
        va_r = v_aug.rearrange("p (s h e) -> p s h e", s=NS, h=H)
        nc.gpsimd.memset(va_r[:, :, :, HS], 1.0)

        # ---------------- qT/kT destinations ----------------
        qT_a = qk_pool.tile([P, T], BF16)       # heads 0..3, d-major
        qT_b = qk_pool.tile([C - P, T], BF16)   # heads 4,5
        kT_a = qk_pool.tile([P, T], BF16)
        kT_b = qk_pool.tile([C - P, T], BF16)

        def hsrc(h):
            if h < 4:
                return kT_a, qT_a, HS * h
            return kT_b, qT_b, HS * (h - 4)

        copy_eng = {
            "a": lambda d, s: nc.scalar.copy(d, s),
            "d": lambda d, s: nc.vector.tensor_copy(d, s),
            "p": lambda d, s: nc.vector.tensor_copy(d, s),
        }

        with (
            tc.tile_pool(name="pstA", bufs=2, space="PSUM") as pstA_pool,
            tc.tile_pool(name="pstD", bufs=2, space="PSUM") as pstD_pool,
            tc.tile_pool(name="pav", bufs=1, space="PSUM") as pav_pool,
        ):
            def proj_pair(proj, c0, eng1, eng2):
                """both dlo chunks of one [*, TCH] column range of qT/kT."""
                ps = pstA_pool.tile([P, 2 * TCH], F32, name="prj", tag="stA")
                dst_a, dst_b = ((qT_a, qT_b) if proj == "q" else (kT_a, kT_b))
                wa, wb = w_sb[proj]
                nc.tensor.matmul(ps[:, 0:TCH], wa[:, 0:P],
                                 xT_a[:, c0:c0 + TCH], start=True, stop=False)
                nc.tensor.matmul(ps[:, 0:TCH], wb[:, 0:P],
                                 xT_b[:, c0:c0 + TCH], start=False, stop=True)
                nc.tensor.matmul(ps[0:C - P, TCH:2 * TCH], wa[:, P:H * HS],
                                 xT_a[:, c0:c0 + TCH], start=True, stop=False)
                nc.tensor.matmul(ps[0:C - P, TCH:2 * TCH], wb[:, P:H * HS],
                                 xT_b[:, c0:c0 + TCH], start=False, stop=True)
                copy_eng[eng1](dst_a[:, c0:c0 + TCH], ps[:, 0:TCH])
                copy_eng[eng2](dst_b[:, c0:c0 + TCH],
                               ps[0:C - P, TCH:2 * TCH])

            def v_chunk(si, eng):
                """v_aug values for one s-tile, via the (idle in tc0) y bank."""
                s0 = si * P
                wva, wvb = w_sb["v"]
                ps = pstD_pool.tile([P, TCH], F32, name="pyv", tag="stD")
                nc.tensor.matmul(ps[:, 0:H * HS], xT_a[:, s0:s0 + P], wva,
                                 start=True, stop=False)
                nc.tensor.matmul(ps[:, 0:H * HS], xT_b[:, s0:s0 + P], wvb,
                                 start=False, stop=True)
                ps_r = ps[:, 0:H * HS].rearrange("p (h d) -> p h d", h=H)
                copy_eng[eng](va_r[:, si, :, 0:HS], ps_r)

            recs = {}

            def post_norm(av, tt):
                """normalize one t-tile: O' = O * recip(rowsum)."""
                av_t = av[tt // 2].rearrange("p (u h e) -> p u h e", u=2, h=H)
                u = tt % 2
                if u == 0:  # one reciprocal covers both u-slots of the bank
                    rec = post_pool.tile([P, 2 * H], F32, name="rec", tag="rec")
                    nc.vector.reciprocal(
                        rec.rearrange("p (u h) -> p u h", u=2),
                        av_t[:, :, :, HS])
                    recs[tt // 2] = rec
                rec = recs[tt // 2].rearrange("p (u h) -> p u h", u=2)[:, u, :]
                onorm = post_pool.tile([P, H * HS], BF16,
                                       name="onorm", tag="onorm")
                on_r = onorm.rearrange("p (h e) -> p h e", h=H)
                nc.vector.tensor_tensor(
                    on_r, av_t[:, u, :, 0:HS],
                    rec.unsqueeze(2).to_broadcast([P, H, HS]),
                    Alu.mult)
                return onorm

            def post_issue(onorm):
                """phase 1: start the two xbar transposes of one t-tile."""
                oT1 = oT_pool.tile([P, P], BF16, name="oT1", tag="oT1")
                nc.sync.dma_start_transpose(oT1, onorm[:, 0:P])
                oT2 = oT_pool.tile([P, P], BF16, name="oT2", tag="oT2")
                nc.sync.dma_start_transpose(oT2, onorm[:, C - P:C])
                return oT1, oT2

            def post_proj(tc0, oTs, tt, pool=None):
                """phase 2: project and store one t-tile (oTs ready-ish, so
                the borrowed psum slot is held briefly)."""
                oT1, oT2 = oTs
                if pool is None or pool is pstD_pool:
                    ps = pstD_pool.tile([P, TCH], F32, name="pyy", tag="stD")
                else:
                    ps = pstA_pool.tile([P, 2 * TCH], F32,
                                        name="pyy2", tag="stA")
                nc.tensor.matmul(ps[:, 0:C], ones1, bp_sb,
                                 start=True, stop=False)
                nc.tensor.matmul(ps[:, 0:C], oT1, wp_a,
                                 start=False, stop=False)
                nc.tensor.matmul(ps[:, 0:C], oT2[C - P:P, :],
                                 wpb_sb[C - P:P, :],
                                 start=False, stop=True)
                ysb = ysb_pool.tile([P, C], F32, name="ysbt", tag="ysbt")
                nc.vector.tensor_copy(ysb, ps[:, 0:C])
                oq = nc.sync if tt % 2 == 0 else nc.gpsimd
                oq.dma_start(out[tc0 + tt * P:tc0 + (tt + 1) * P, :], ysb)

            def emit_av(av, si, ptiles, heads, tts=tuple(range(NT))):
                """bf16 AV for the non-ACT heads of one s-tile."""
                for h in heads:
                    for tt in tts:
                        av_t = av[tt // 2].rearrange(
                            "p (u h e) -> p u h e", u=2, h=H)
                        nc.tensor.matmul(
                            av_t[:, tt % 2, h, :],
                            ptiles[h][:, tt * P:(tt + 1) * P],
                            va_r[:, si, h, :],
                            start=(si == 0 and h == heads[0]
                                   and tt % 2 == 0),
                            stop=(si == NS - 1),
                            skip_group_check=True)


            def qkt(st_dst, h, si, tc0):
                kT_t, qT_t, pb = hsrc(h)
                nc.tensor.matmul(
                    st_dst, kT_t[pb:pb + HS, si * P:si * P + P],
                    qT_t[pb:pb + HS, tc0:tc0 + TCH],
                    start=True, stop=True, tile_position=(pb, 0))

            # head-group -> engine rotation per si: each group of 2 heads goes
            # to one of ACT / DVE / Pool(2 singles)
            GROUPS = ((0, 1), (2, 3), (4, 5))

            # prologue. k-mini first: si-0 QKTs only need kT cols 0:128, so a
            # tiny k chunk unblocks them well before the full k pair lands
            kmini = pstD_pool.tile([P, TCH], F32, name="kmini", tag="stD")
            wka, wkb = w_sb["k"]
            nc.tensor.matmul(kmini[:, 0:P], wka[:, 0:P], xT_a[:, 0:P],
                             start=True, stop=False)
            nc.tensor.matmul(kmini[:, 0:P], wkb[:, 0:P], xT_b[:, 0:P],
                             start=False, stop=True)
            nc.tensor.matmul(kmini[0:C - P, P:2 * P], wka[:, P:H * HS],
                             xT_a[:, 0:P], start=True, stop=False)
            nc.tensor.matmul(kmini[0:C - P, P:2 * P], wkb[:, P:H * HS],
                             xT_b[:, 0:P], start=False, stop=True)
            nc.vector.tensor_copy(kT_a[:, 0:P], kmini[:, 0:P])
            nc.vector.tensor_copy(kT_b[:, 0:P], kmini[0:C - P, P:2 * P])
            # q pair with copies split across engines for latency
            q_ps = pstA_pool.tile([P, 2 * TCH], F32, name="qprl", tag="stA")
            wqa, wqb = w_sb["q"]
            nc.tensor.matmul(q_ps[:, 0:TCH], wqa[:, 0:P],
                             xT_a[:, 0:TCH], start=True, stop=False)
            nc.tensor.matmul(q_ps[:, 0:TCH], wqb[:, 0:P],
                             xT_b[:, 0:TCH], start=False, stop=True)
            nc.tensor.matmul(q_ps[0:C - P, TCH:2 * TCH], wqa[:, P:H * HS],
                             xT_a[:, 0:TCH], start=True, stop=False)
            nc.tensor.matmul(q_ps[0:C - P, TCH:2 * TCH], wqb[:, P:H * HS],
                             xT_b[:, 0:TCH], start=False, stop=True)
            nc.scalar.copy(qT_a[:, 0:2 * P], q_ps[:, 0:2 * P])
            nc.vector.tensor_copy(qT_a[:, 2 * P:TCH], q_ps[:, 2 * P:TCH])
            nc.vector.tensor_copy(qT_b[:, 0:TCH],
                                  q_ps[0:C - P, TCH:2 * TCH])
            # k for the rest of the first s-range (cols 128:512 — the kmini
            # already covered 0:128)
            k_ps = pstA_pool.tile([P, 2 * TCH], F32, name="kprl", tag="stA")
            nc.tensor.matmul(k_ps[:, P:TCH], wka[:, 0:P],
                             xT_a[:, P:TCH], start=True, stop=False)
            nc.tensor.matmul(k_ps[:, P:TCH], wkb[:, 0:P],
                             xT_b[:, P:TCH], start=False, stop=True)
            nc.tensor.matmul(k_ps[0:C - P, TCH + P:2 * TCH], wka[:, P:H * HS],
                             xT_a[:, P:TCH], start=True, stop=False)
            nc.tensor.matmul(k_ps[0:C - P, TCH + P:2 * TCH], wkb[:, P:H * HS],
                             xT_b[:, P:TCH], start=False, stop=True)
            nc.scalar.copy(kT_a[:, P:TCH], k_ps[:, P:TCH])
            nc.vector.tensor_copy(kT_b[:, P:TCH],
                                  k_ps[0:C - P, TCH + P:2 * TCH])

            # deferred projection chunks, keyed by (tc index, si)
            deferred = {}
            for i, c0 in enumerate((TCH, 2 * TCH, 3 * TCH)):
                deferred.setdefault((0, 3 * i + 1), []).append(
                    ("k", c0, "a", "d"))
            for tci in range(NT - 1):
                deferred.setdefault((tci, 9), []).append(
                    ("q", (tci + 1) * TCH, "a", "d"))

            prev1 = prev2 = prev3 = None  # (av, si, ...): AV lags 3 si
            dr_q = []             # [(j, ptJ, ga)] DoubleRow AV queue
            cur_ptJ = None
            pending = None  # (tc0, onorms) awaiting post_proj
            av = None
            for tci, tc0 in enumerate(range(0, T, TCH)):
                old_av = av
                av = [pav_pool.tile([P, 2 * H * E1], F32,
                                    name=f"av{i}", tag=f"av{i}")
                      for i in range(2)]
                for si in range(NS):
                    rot = (si // 2) % 3  # per s-tile PAIR so the ACT group
                    # (fp8 DoubleRow) is uniform across the pair
                    eng_of = {g: ("a", "d", "p")[(g + rot) % 3]
                              for g in range(3)}
                    ptiles = [None] * H
                    ga = next(g for g in range(3) if eng_of[g] == "a")
                    gd = next(g for g in range(3) if eng_of[g] == "d")
                    gp = next(g for g in range(3) if eng_of[g] == "p")
                    heads_bf = list(range(H))

                    # ACT group -> one [128,1024] exp into the fp8 pair
                    # container (DoubleRow AV); stA pair tiles bufs=2
                    h_p0, h_p1 = GROUPS[gp]
                    stA = pstA_pool.tile([P, 2 * TCH], F32, name="stA",
                                         tag="stA")
                    for half, h in enumerate(GROUPS[ga]):
                        qkt(stA[:, half * TCH:(half + 1) * TCH], h, si, tc0)
                    ptpA = ptA_pool.tile([P, 2 * TCH], BF16, name="ptpA",
                                         tag="ptj")
                    nc.scalar.activation(ptpA, stA, Exp, scale=SCALE,
                                         bias=bias_act)
                    ptiles[GROUPS[ga][0]] = ptpA[:, 0:TCH]
                    ptiles[GROUPS[ga][1]] = ptpA[:, TCH:2 * TCH]

                    # gp group: ACT bf16 pair on 7 of 8 si, DVE singles else
                    if si % 8 != 7 and si % 32 != 3:
                        stG = pstA_pool.tile([P, 2 * TCH], F32, name="stG",
                                             tag="stA")
                        for half, h in enumerate(GROUPS[gp]):
                            qkt(stG[:, half * TCH:(half + 1) * TCH],
                                h, si, tc0)
                        ptpG = ptP_pool.tile([P, 2 * TCH], BF16, name="ptpG",
                                             tag="ptp")
                        nc.scalar.activation(ptpG, stG, Exp, scale=SCALE,
                                             bias=bias_act)
                        ptiles[h_p0] = ptpG[:, 0:TCH]
                        ptiles[h_p1] = ptpG[:, TCH:2 * TCH]
                    else:
                        for h in GROUPS[gp]:
                            stG = pstD_pool.tile([P, TCH], F32, name="stG1",
                                                 tag="stD")
                            qkt(stG, h, si, tc0)
                            ptpG = ptP_pool.tile([P, 2 * TCH], BF16,
                                                 name="ptpG", tag="ptp")
                            nc.vector.tensor_scalar(
                                ptpG.bitcast(I16)[:, 0:TCH], stG,
                                S1, S2, Alu.mult, Alu.add)
                            ptiles[h] = ptpG[:, 0:TCH]

                    # DVE singles, double-buffered via pstD bufs=2
                    for h in GROUPS[gd]:
                        stD = pstD_pool.tile([P, TCH], F32, name="stD",
                                             tag="stD")
                        qkt(stD, h, si, tc0)
                        ptpD = ptD_pool.tile([P, TCH], BF16, name="ptpD",
                                             tag="ptpd")
                        nc.vector.tensor_scalar(ptpD.bitcast(I16), stD,
                                                S1, S2, Alu.mult, Alu.add)
                        ptiles[h] = ptpD

                    for args in deferred.get((tci, si), ()):
                        proj_pair(*args)
                    if tci == 0:
                        v_chunk(si, "d")
                    if si == 0 and old_av is not None:
                        # flush prev tc: last three s-tiles
                        emit_av(old_av, NS - 3, prev3[2], prev3[3])
                        emit_av(old_av, NS - 2, prev2[2], prev2[3])
                        emit_av(old_av, NS - 1, prev1[2], prev1[3])
                        prev3 = prev2 = prev1 = None
                        pending = (tc0 - TCH,
                                   [post_norm(old_av, t) for t in (0, 1)])
                    if si == 1 and pending is not None and len(pending[1]) == 2:
                        pending[1].extend(post_norm(old_av, t) for t in (2, 3))
                    if si % 4 == 2 and pending is not None:
                        post_proj(pending[0],
                                  post_issue(pending[1][si // 4]), si // 4)
                        if si // 4 == NT - 1:
                            pending = None
                    if prev3 is not None:
                        emit_av(av, prev3[1], prev3[2], prev3[3])
                    prev3 = prev2
                    prev2 = prev1
                    prev1 = (av, si, ptiles, heads_bf)

            # tail: drain av bank 0 fully first so its norms/projections
            # overlap the remaining AV matmuls of bank 1
            tail_pool = (None, pstA_pool, pstD_pool, None)
            emit_av(av, NS - 3, prev3[2], prev3[3], tts=(0, 1))
            emit_av(av, NS - 2, prev2[2], prev2[3], tts=(0, 1))
            emit_av(av, NS - 1, prev1[2], prev1[3], tts=(0, 1))
            oT0 = post_issue(post_norm(av, 0))
            oT1t = post_issue(post_norm(av, 1))
            post_proj(tc0, oT0, 0, pool=tail_pool[0])
            emit_av(av, NS - 3, prev3[2], prev3[3], tts=(2, 3))
            emit_av(av, NS - 2, prev2[2], prev2[3], tts=(2, 3))
            emit_av(av, NS - 1, prev1[2], prev1[3], tts=(2, 3))
            post_proj(tc0, oT1t, 1, pool=tail_pool[1])
            oT2t = post_issue(post_norm(av, 2))
            oT3t = post_issue(post_norm(av, 3))
            post_proj(tc0, oT2t, 2, pool=tail_pool[2])
            post_proj(tc0, oT3t, 3, pool=tail_pool[3])

    nc.compile()
    return nc


def _get_nc():
    if "nc" not in _CACHE:
        _CACHE["nc"] = build_nc()
    return _CACHE["nc"]


def make_in_maps(x, Wq, Wk, Wv, Wproj, bproj):
    bf = ml_dtypes.bfloat16
    f8 = ml_dtypes.float8_e4m3
    x = np.asarray(x, np.float32)
    pack = lambda w: np.ascontiguousarray(
        np.transpose(np.asarray(w, np.float32), (1, 0, 2)).reshape(C, H * HS)
    ).astype(bf)
    wq, wk, wv = pack(Wq), pack(Wk), pack(Wv)
    wp = np.ascontiguousarray(
        np.asarray(Wproj, np.float32).reshape(H * HS, C)).astype(bf)
    bp = np.asarray(bproj, np.float32).reshape(1, C).astype(bf)
    maps = []
    for i in range(B):
        xti = np.ascontiguousarray(x[i].T).astype(bf)
        maps.append({"xT": xti, "wq": wq, "wk": wk, "wv": wv,
                     "wp": wp, "bp": bp})
    return maps


def run(inputs, trace=False, **kw):
    nc = _get_nc()
    in_maps = make_in_maps(**inputs)
    res = run_bass_kernel_spmd(nc, in_maps, core_ids=list(range(B)),
                               trace=trace, **kw)
    y = np.stack([np.asarray(res.results[i]["out"], np.float32)
                  for i in range(B)], axis=0)
    return y, res


def kernel(**inputs):
    y, _ = run(inputs, trace=False)
    return y
